# revision 22
# baseline (speedup 1.0000x reference)
"""Trainium2 Bass kernel for nn_BinConv2d: BN(train-mode) -> sign -> 3x3 conv.

Single fused launch per core (8 cores, batch-sharded; core c owns images
[2c, 2c+1] viewed as [128 partitions = 2 img x 64 ch, 224, 224]):

  Phase 1 (stats, DMA-bound): x f32 is DMA'd in with an inline cast to a
    persistent fp16 SBUF copy (12.85 MB, fits) while bn_stats/bn_aggr build
    per-(img,ch) mean/var.  Per-partition [mean, E[x^2]] are AllReduce'd
    across the 8 cores (1 KB, ~10 us), the two image halves are folded
    on-device, and BN+sign collapse into a per-channel threshold:
        sign(BN(x)) = sign(x - t),  t = m - (bias/weight)*sqrt(var+eps)
    (bn_weight > 0).  bias/weight is precomputed on host.

  Phase 2 (conv, PE-bound): per 28-row band, ACT computes Sign(x16 - t)
    into a width-padded fp16 strip; a SBUF->SBUF DMA builds a copy of the
    strip shifted down one row on the other partition half, so one matmul
    contracts 128 partitions = 2 vertical taps x 64 ch.  The 9 taps become
    3 pair-rounds (dy=0&1) + 3 single-rounds (dy=2); each round runs img0
    (psum partitions 0:64) and img1 (64:128) concurrently on disjoint PE
    column groups, so pair-rounds use the full 128x128 array.  PSUM tiles
    cycle through banks in groups of 4; DVE evacuates (+conv bias), DMA out.

  Weights fp16 (activations are exactly {-1,0,+1}), PSUM accumulation f32.
  The fp16 storage of x only perturbs sign(x - t) for |x - t| < ~1 fp16 ulp.
"""

import sys

if "/opt/trn_rl_repo" not in sys.path:
    sys.path.insert(0, "/opt/trn_rl_repo")

import numpy as np

import concourse.bacc as bacc
import concourse.tile as tile
from concourse import mybir
from concourse.bass_utils import run_bass_kernel_spmd

F32 = mybir.dt.float32
F16 = mybir.dt.float16

N_CORES = 8
N, C, H, W = 16, 64, 224, 224
BN_EPS = 1e-4
BAND = 28              # output rows per band
NB = H // BAND         # 8 bands
WP = W + 2             # padded strip width (226)
NT = BAND // 2         # 14 tiles per band, 2 output rows (452 cols) each
MM_N = 2 * WP          # 452 matmul free dim
STRIP_ROWS = BAND + 2  # 30
PSTRIP = STRIP_ROWS * WP + 2   # padded strip buffer len (6782)
HW_ELEMS = H * W       # 50176
N_GROUPS = HW_ELEMS // 512     # 98 bn_stats groups
CHUNK_GROUPS = [25, 25, 24, 24]  # phase-1 cast-DMA chunks (x512 elems)


def build_fused_nc(repeat=1, timing=None, mm_variant=None):
    """timing=None: the real kernel (external x/y, collective, repeat must
    be 1).  timing=(phase, R): internal x/y + tiny output, measuring `R`
    hardware-loop iterations of 'iter' (phase1+phase2, no collective),
    'p1', or 'p2'."""
    nc = bacc.Bacc(num_devices=N_CORES)
    if timing is not None:
        x_s = nc.dram_tensor("x_int", [128, H * W], F32)
        y = nc.dram_tensor("y_int", [128, H, W], F32)
        tout = nc.declare_dram_parameter("tout", [128, 2], F32, isOutput=True)
    else:
        x_s = nc.declare_dram_parameter("x_s", [128, H * W], F32, isOutput=False)
        y = nc.declare_dram_parameter("y", [128, H, W], F32, isOutput=True)
        tout = None
    wts = nc.declare_dram_parameter("wts", [128, 9, C], F16, isOutput=False)
    bw = nc.declare_dram_parameter("bw", [64, 1], F32, isOutput=False)
    cbias = nc.declare_dram_parameter("cbias", [128, 1], F32, isOutput=False)

    with tile.TileContext(nc) as tc:
        with (
            tc.tile_pool(name="const", bufs=1) as cpool,
            tc.tile_pool(name="stats", bufs=1) as tpool,
            tc.tile_pool(name="stage", bufs=2) as opool,
            tc.tile_pool(name="psum", bufs=8, space="PSUM") as ppool,
            tc.tile_pool(name="dram", bufs=1, space="DRAM") as dram,
        ):
            wsb = cpool.tile([128, 9, C], F16)
            nc.sync.dma_start(out=wsb[:], in_=wts[:])
            bwsb = cpool.tile([64, 1], F32)
            nc.sync.dma_start(out=bwsb[:], in_=bw[:])
            bsb = cpool.tile([128, 1], F32)
            nc.sync.dma_start(out=bsb[:], in_=cbias[:])

            # persistent fp16 copy of x
            x16 = cpool.tile([128, H, W], F16, name="x16")
            x16f = x16[:].rearrange("p a b -> p (a b)")

            # strip-pair buffers: SP0 (img0): [0:64]=strip, [64:128]=strip
            # shifted down one row; SP1 (img1): halves swapped.  Double
            # buffered across bands.
            sp0 = [cpool.tile([128, PSTRIP], F16, name=f"sp0_{i}") for i in (0, 1)]
            sp1 = [cpool.tile([128, PSTRIP], F16, name=f"sp1_{i}") for i in (0, 1)]
            for s in sp0 + sp1:
                nc.vector.memset(s[:], 0.0)

            tneg = cpool.tile([128, 1], F32, name="tneg")

            def emit_stats(it):
                stats = tpool.tile([128, N_GROUPS, 6], F32, tag="stats")
                mv = tpool.tile([128, 2], F32, tag="mv")
                g0 = 0
                for ng in CHUNK_GROUPS:
                    e0, e1 = g0 * 512, (g0 + ng) * 512
                    nc.gpsimd.dma_start(
                        out=x16f[:, e0:e1], in_=x_s[:, e0:e1]
                    )
                    for g in range(g0, g0 + ng):
                        nc.vector.bn_stats(
                            out=stats[:, g, :],
                            in_=x16f[:, g * 512 : (g + 1) * 512],
                        )
                    g0 += ng
                nc.vector.bn_aggr(out=mv[:], in_=stats[:])

                # u = [mean, var + mean^2] per partition
                u = tpool.tile([128, 2], F32, tag="u")
                nc.vector.tensor_copy(out=u[:, 0:1], in_=mv[:, 0:1])
                nc.vector.tensor_tensor(
                    out=u[:, 1:2], in0=mv[:, 0:1], in1=mv[:, 0:1],
                    op=mybir.AluOpType.mult,
                )
                nc.vector.tensor_tensor(
                    out=u[:, 1:2], in0=u[:, 1:2], in1=mv[:, 1:2],
                    op=mybir.AluOpType.add,
                )
                return u

            def emit_cc(it, u):
                in_b = dram.tile([128, 2], F32, tag="ccin")
                out_b = dram.tile([128, 2], F32, tag="ccout")
                nc.gpsimd.dma_start(out=in_b[:], in_=u[:])
                nc.gpsimd.collective_compute(
                    "AllReduce",
                    mybir.AluOpType.add,
                    replica_groups=[list(range(N_CORES))],
                    ins=[in_b[:].opt()],
                    outs=[out_b[:].opt()],
                )
                g = tpool.tile([128, 2], F32, tag="g")
                nc.gpsimd.dma_start(out=g[:], in_=out_b[:])
                # fold image halves: s = g[0:64] + g[64:128]
                h = tpool.tile([64, 2], F32, tag="h")
                nc.sync.dma_start(out=h[:], in_=g[64:128, :])
                s = tpool.tile([64, 2], F32, tag="s")
                nc.vector.tensor_tensor(
                    out=s[:], in0=g[0:64, :], in1=h[:], op=mybir.AluOpType.add
                )
                # m = s0/16 ; var = s1/16 - m^2 ; tneg = bw*sqrt(var+eps) - m
                m = tpool.tile([64, 1], F32, tag="m")
                nc.vector.tensor_scalar(
                    out=m[:], in0=s[:, 0:1], scalar1=1.0 / N, scalar2=None,
                    op0=mybir.AluOpType.mult,
                )
                var = tpool.tile([64, 1], F32, tag="var")
                nc.vector.tensor_scalar(
                    out=var[:], in0=s[:, 1:2], scalar1=1.0 / N, scalar2=None,
                    op0=mybir.AluOpType.mult,
                )
                m2 = tpool.tile([64, 1], F32, tag="m2")
                nc.vector.tensor_tensor(
                    out=m2[:], in0=m[:], in1=m[:], op=mybir.AluOpType.mult
                )
                nc.vector.tensor_tensor(
                    out=var[:], in0=var[:], in1=m2[:],
                    op=mybir.AluOpType.subtract,
                )
                eps = tpool.tile([64, 1], F32, tag="eps")
                nc.vector.memset(eps[:], float(BN_EPS))
                sd = tpool.tile([64, 1], F32, tag="sd")
                nc.scalar.activation(
                    out=sd[:], in_=var[:],
                    func=mybir.ActivationFunctionType.Sqrt, bias=eps[:],
                )
                nc.vector.tensor_tensor(
                    out=sd[:], in0=sd[:], in1=bwsb[:], op=mybir.AluOpType.mult
                )
                nc.vector.tensor_tensor(
                    out=tneg[0:64, :], in0=sd[:], in1=m[:],
                    op=mybir.AluOpType.subtract,
                )
                nc.sync.dma_start(out=tneg[64:128, :], in_=tneg[0:64, :])

            def emit_phase2(it):
                for b in range(NB):
                    r0 = b * BAND
                    lo = max(r0 - 1, 0)
                    hi = min(r0 + BAND + 1, H)
                    s0 = lo - (r0 - 1)
                    nr = hi - lo
                    p0 = sp0[b % 2]
                    p1 = sp1[b % 2]
                    s3_0 = p0[:, 1 : 1 + STRIP_ROWS * WP].rearrange(
                        "p (a b) -> p a b", b=WP
                    )
                    s3_1 = p1[:, 1 : 1 + STRIP_ROWS * WP].rearrange(
                        "p (a b) -> p a b", b=WP
                    )
                    if b == NB - 1:
                        # bottom pad row on the unshifted halves
                        nc.vector.memset(s3_0[0:64, STRIP_ROWS - 1, :], 0.0)
                        nc.vector.memset(s3_1[64:128, STRIP_ROWS - 1, :], 0.0)
                    # sign strips (unshifted halves), emitted in two row
                    # chunks so the first psum group's matmuls can start
                    # before the whole band is signed
                    mid = s0 + nr // 2
                    for (ra, rb) in ((s0, mid), (mid, s0 + nr)):
                        xa, xb_ = lo + (ra - s0), lo + (rb - s0)
                        nc.scalar.activation(
                            out=s3_0[0:64, ra:rb, 1 : 1 + W],
                            in_=x16[0:64, xa:xb_, :],
                            func=mybir.ActivationFunctionType.Sign,
                            bias=tneg[0:64, :],
                        )
                        nc.scalar.activation(
                            out=s3_1[64:128, ra:rb, 1 : 1 + W],
                            in_=x16[64:128, xa:xb_, :],
                            func=mybir.ActivationFunctionType.Sign,
                            bias=tneg[64:128, :],
                        )
                    # shifted copies on the other partition half:
                    # slot s <- strip row s+1, s = 0..28; two chunks each
                    for (sa, sb) in ((0, 14), (14, STRIP_ROWS - 1)):
                        cl = (sb - sa) * WP
                        nc.scalar.dma_start(
                            out=p0[64:128, 1 + sa * WP : 1 + sa * WP + cl],
                            in_=p0[0:64, 1 + (sa + 1) * WP : 1 + (sa + 1) * WP + cl],
                        )
                        nc.scalar.dma_start(
                            out=p1[0:64, 1 + sa * WP : 1 + sa * WP + cl],
                            in_=p1[64:128, 1 + (sa + 1) * WP : 1 + (sa + 1) * WP + cl],
                        )

                    for g0 in range(0, NT, 4):
                        tiles = list(range(g0, min(g0 + 4, NT)))
                        stg = opool.tile([128, 8, W], F32, tag="stg")
                        psums = {
                            i: ppool.tile([128, 512], F32, name=f"ps{i}", tag="ps")
                            for i in tiles
                        }
                        NM = 56 if mm_variant == "n56" else MM_N
                        # pair rounds: taps (0,dx)+(1,dx) via 128-row
                        # contraction (strip + shifted strip); full array
                        # per span (img0 on psum 0:64 / col grps 0-1, img1
                        # on 64:128 / col grps 2-3).
                        for dx in range(3):
                            wdx0 = 0 if mm_variant == "samew" else dx
                            wdx1 = 0 if mm_variant == "samew" else 3 + dx
                            for i in tiles:
                                st = (2 * i) * WP + dx
                                nc.tensor.matmul(
                                    psums[i][0:64, :NM],
                                    wsb[:, wdx0, :],
                                    p0[:, st : st + NM],
                                    start=(dx == 0), stop=False,
                                    skip_group_check=True,
                                )
                                nc.tensor.matmul(
                                    psums[i][64:128, :NM],
                                    wsb[:, wdx1, :],
                                    p1[:, st : st + NM],
                                    start=(dx == 0), stop=False,
                                    skip_group_check=True,
                                )
                        # single rounds: tap (2,dx), K=64.  Two tiles share
                        # one span: tile i streams the unshifted half (row
                        # grps of that half), tile j=i+1 streams the other
                        # half via the shifted copy at st-WP; disjoint
                        # 32x32 subarray sets -> concurrent.
                        for dx in range(3):
                            last = dx == 2
                            wdxs = 0 if mm_variant == "samew" else 6 + dx
                            for k in range(0, len(tiles), 2):
                                i = tiles[k]
                                sti = (2 * i + 2) * WP + dx
                                nc.tensor.matmul(
                                    psums[i][0:64, :NM],
                                    wsb[0:64, wdxs, :],
                                    p0[0:64, sti : sti + NM],
                                    start=False, stop=last,
                                    skip_group_check=True,
                                )
                                nc.tensor.matmul(
                                    psums[i][64:128, :NM],
                                    wsb[64:128, wdxs, :],
                                    p1[64:128, sti : sti + NM],
                                    start=False, stop=last,
                                    skip_group_check=True,
                                )
                                if k + 1 < len(tiles):
                                    j = tiles[k + 1]
                                    stj = (2 * j + 1) * WP + dx
                                    nc.tensor.matmul(
                                        psums[j][0:64, :NM],
                                        wsb[64:128, wdxs, :],
                                        p0[64:128, stj : stj + NM],
                                        start=False, stop=last,
                                        skip_group_check=True,
                                    )
                                    nc.tensor.matmul(
                                        psums[j][64:128, :NM],
                                        wsb[0:64, wdxs, :],
                                        p1[0:64, stj : stj + NM],
                                        start=False, stop=last,
                                        skip_group_check=True,
                                    )
                        for k, i in enumerate(tiles):
                            ps3 = psums[i][:, :MM_N].rearrange(
                                "p (r c) -> p r c", c=WP
                            )
                            nc.vector.tensor_scalar(
                                out=stg[:, 2 * k : 2 * k + 2, :],
                                in0=ps3[:, :, 1 : 1 + W],
                                scalar1=bsb[:],
                                scalar2=None,
                                op0=mybir.AluOpType.add,
                            )
                        nc.sync.dma_start(
                            out=y[:, r0 + 2 * g0 : r0 + 2 * g0 + 2 * len(tiles), :],
                            in_=stg[:, : 2 * len(tiles), :],
                        )

            if timing is None:
                for it in range(repeat):
                    u = emit_stats(it)
                    emit_cc(it, u)
                    emit_phase2(it)
            else:
                phase, R = timing
                mvout = tpool.tile([128, 2], F32, tag="mvout")
                nc.vector.memset(mvout[:], 0.0)
                if phase != "p1":
                    nc.vector.memset(tneg[:], 0.0)
                nc.vector.memset(x16[:], 0.0)
                if phase == "iter":
                    with tc.For_i(0, R, 1):
                        u = emit_stats(0)
                        emit_phase2(0)
                elif phase == "p1":
                    with tc.For_i(0, R, 1):
                        u = emit_stats(0)
                elif phase == "p2":
                    with tc.For_i(0, R, 1):
                        emit_phase2(0)
                elif phase == "cc":
                    for it in range(R):
                        u = emit_stats(it) if it == 0 else u
                        emit_cc(it, u)
                else:
                    raise ValueError(phase)
                nc.sync.dma_start(out=tout[:], in_=mvout[:])
    nc.compile()
    return nc


_cache = {}


def _get(name, builder):
    if name not in _cache:
        _cache[name] = builder()
    return _cache[name]


def _prep_inputs(bn_weight, bn_bias, conv_weight, conv_bias):
    # lhsT layouts; w9[cin, dy, dx, cout]
    w9 = conv_weight.transpose(1, 2, 3, 0).reshape(C, 3, 3, C)
    wts = np.empty((128, 9, C), np.float16)
    for dx in range(3):
        wts[0:64, dx, :] = w9[:, 0, dx, :]       # img0 pair: strip half
        wts[64:128, dx, :] = w9[:, 1, dx, :]     # img0 pair: shifted half
        wts[0:64, 3 + dx, :] = w9[:, 1, dx, :]   # img1 pair: shifted half
        wts[64:128, 3 + dx, :] = w9[:, 0, dx, :]  # img1 pair: strip half
        wts[0:64, 6 + dx, :] = w9[:, 2, dx, :]   # singles
        wts[64:128, 6 + dx, :] = w9[:, 2, dx, :]
    bwv = (bn_bias.astype(np.float64) / bn_weight.astype(np.float64)).astype(
        np.float32
    )[:, None]
    cb = np.tile(conv_bias.astype(np.float32), 2)[:, None]
    return wts, bwv, cb


def kernel(x, bn_weight, bn_bias, conv_weight, conv_bias):
    x = np.ascontiguousarray(np.asarray(x), dtype=np.float32)
    bn_weight = np.asarray(bn_weight, dtype=np.float32)
    bn_bias = np.asarray(bn_bias, dtype=np.float32)
    conv_weight = np.asarray(conv_weight, dtype=np.float32)
    conv_bias = np.asarray(conv_bias, dtype=np.float32)

    wts, bwv, cb = _prep_inputs(bn_weight, bn_bias, conv_weight, conv_bias)

    ipc = N // N_CORES
    nc = _get("fused", build_fused_nc)
    in_maps = [
        {
            "x_s": x[ipc * c : ipc * (c + 1)].reshape(128, H * W),
            "wts": wts,
            "bw": bwv,
            "cbias": cb,
        }
        for c in range(N_CORES)
    ]
    res = run_bass_kernel_spmd(nc, in_maps, list(range(N_CORES))).results
    out = np.concatenate(
        [res[c]["y"].reshape(ipc, C, H, W) for c in range(N_CORES)], axis=0
    )
    return out


# revision 23
# speedup vs baseline: 1.3795x; 1.3795x over previous
"""Trainium2 Bass kernel for nn_BinConv2d: BN(train-mode) -> sign -> 3x3 conv.

Single fused launch per core (8 cores, batch-sharded; core c owns images
[2c, 2c+1] viewed as [128 partitions = 2 img x 64 ch, 224, 224]):

  Phase 1 (stats, DMA-bound): x f32 is DMA'd in with an inline cast to a
    persistent fp16 SBUF copy (12.85 MB, fits) while bn_stats/bn_aggr build
    per-(img,ch) mean/var.  Per-partition [mean, E[x^2]] are AllReduce'd
    across the 8 cores (1 KB, ~10 us), the two image halves are folded
    on-device, and BN+sign collapse into a per-channel threshold:
        sign(BN(x)) = sign(x - t),  t = m - (bias/weight)*sqrt(var+eps)
    (bn_weight > 0).  bias/weight is precomputed on host.

  Phase 2 (conv, PE-bound): per 28-row band, ACT computes Sign(x16 - t)
    into a width-padded fp16 strip; a SBUF->SBUF DMA builds a copy of the
    strip shifted down one row on the other partition half, so one matmul
    contracts 128 partitions = 2 vertical taps x 64 ch.  The 9 taps become
    3 pair-rounds (dy=0&1) + 3 single-rounds (dy=2); each round runs img0
    (psum partitions 0:64) and img1 (64:128) concurrently on disjoint PE
    column groups, so pair-rounds use the full 128x128 array.  PSUM tiles
    cycle through banks in groups of 4; DVE evacuates (+conv bias), DMA out.

  Weights fp16 (activations are exactly {-1,0,+1}), PSUM accumulation f32.
  The fp16 storage of x only perturbs sign(x - t) for |x - t| < ~1 fp16 ulp.
"""

import sys

if "/opt/trn_rl_repo" not in sys.path:
    sys.path.insert(0, "/opt/trn_rl_repo")

import numpy as np

import concourse.bacc as bacc
import concourse.tile as tile
from concourse import mybir
from concourse.bass_utils import run_bass_kernel_spmd

F32 = mybir.dt.float32
F16 = mybir.dt.float16

N_CORES = 8
N, C, H, W = 16, 64, 224, 224
BN_EPS = 1e-4
BAND = 28              # output rows per band
NB = H // BAND         # 8 bands
WP = W + 2             # padded strip width (226)
NT = BAND // 2         # 14 tiles per band, 2 output rows (452 cols) each
MM_N = 2 * WP          # 452 matmul free dim
STRIP_ROWS = BAND + 2  # 30
PSTRIP = STRIP_ROWS * WP + 2   # padded strip buffer len (6782)
HW_ELEMS = H * W       # 50176
N_GROUPS = HW_ELEMS // 512     # 98 bn_stats groups
# phase-1 cast-DMA chunks (units of 512 elems); the last chunk is small so
# the bn_stats tail after the final DMA byte is short
CHUNK_GROUPS = [30, 30, 30, 8]


def build_fused_nc(repeat=1, timing=None, mm_variant=None):
    """timing=None: the real kernel (external x/y, collective, repeat must
    be 1).  timing=(phase, R): internal x/y + tiny output, measuring `R`
    hardware-loop iterations of 'iter' (phase1+phase2, no collective),
    'p1', or 'p2'."""
    nc = bacc.Bacc(num_devices=N_CORES)
    if timing is not None:
        x_s = nc.dram_tensor("x_int", [128, H * W], F32)
        y = nc.dram_tensor("y_int", [128, H, W], F32)
        tout = nc.declare_dram_parameter("tout", [128, 2], F32, isOutput=True)
    else:
        x_s = nc.declare_dram_parameter("x_s", [128, H * W], F32, isOutput=False)
        y = nc.declare_dram_parameter("y", [128, H, W], F32, isOutput=True)
        tout = None
    wts = nc.declare_dram_parameter("wts", [128, 9, C], F16, isOutput=False)
    bw = nc.declare_dram_parameter("bw", [64, 1], F32, isOutput=False)
    cbias = nc.declare_dram_parameter("cbias", [128, 1], F32, isOutput=False)

    with tile.TileContext(nc) as tc:
        with (
            tc.tile_pool(name="const", bufs=1) as cpool,
            tc.tile_pool(name="stats", bufs=1) as tpool,
            tc.tile_pool(name="stage", bufs=2) as opool,
            tc.tile_pool(name="psum", bufs=8, space="PSUM") as ppool,
            tc.tile_pool(name="dram", bufs=1, space="DRAM") as dram,
        ):
            wsb = cpool.tile([128, 9, C], F16)
            nc.sync.dma_start(out=wsb[:], in_=wts[:])
            bwsb = cpool.tile([64, 1], F32)
            nc.sync.dma_start(out=bwsb[:], in_=bw[:])
            bsb = cpool.tile([128, 1], F32)
            nc.sync.dma_start(out=bsb[:], in_=cbias[:])

            # persistent fp16 copy of x
            x16 = cpool.tile([128, H, W], F16, name="x16")
            x16f = x16[:].rearrange("p a b -> p (a b)")

            # strip-pair buffers: SP0 (img0): [0:64]=strip, [64:128]=strip
            # shifted down one row; SP1 (img1): halves swapped.  Double
            # buffered across bands.
            sp0 = [cpool.tile([128, PSTRIP], F16, name=f"sp0_{i}") for i in (0, 1)]
            sp1 = [cpool.tile([128, PSTRIP], F16, name=f"sp1_{i}") for i in (0, 1)]
            for s in sp0 + sp1:
                nc.vector.memset(s[:], 0.0)

            tneg = cpool.tile([128, 1], F32, name="tneg")

            def emit_stats(it):
                stats = tpool.tile([128, N_GROUPS, 6], F32, tag="stats")
                mv = tpool.tile([128, 2], F32, tag="mv")
                g0 = 0
                for ng in CHUNK_GROUPS:
                    e0, e1 = g0 * 512, (g0 + ng) * 512
                    nc.gpsimd.dma_start(
                        out=x16f[:, e0:e1], in_=x_s[:, e0:e1]
                    )
                    for g in range(g0, g0 + ng):
                        nc.vector.bn_stats(
                            out=stats[:, g, :],
                            in_=x16f[:, g * 512 : (g + 1) * 512],
                        )
                    g0 += ng
                nc.vector.bn_aggr(out=mv[:], in_=stats[:])

                # u = [mean, var + mean^2] per partition
                u = tpool.tile([128, 2], F32, tag="u")
                nc.vector.tensor_copy(out=u[:, 0:1], in_=mv[:, 0:1])
                nc.vector.tensor_tensor(
                    out=u[:, 1:2], in0=mv[:, 0:1], in1=mv[:, 0:1],
                    op=mybir.AluOpType.mult,
                )
                nc.vector.tensor_tensor(
                    out=u[:, 1:2], in0=u[:, 1:2], in1=mv[:, 1:2],
                    op=mybir.AluOpType.add,
                )
                return u

            def emit_cc(it, u):
                in_b = dram.tile([128, 2], F32, tag="ccin")
                out_b = dram.tile([128, 2], F32, tag="ccout")
                nc.gpsimd.dma_start(out=in_b[:], in_=u[:])
                nc.gpsimd.collective_compute(
                    "AllReduce",
                    mybir.AluOpType.add,
                    replica_groups=[list(range(N_CORES))],
                    ins=[in_b[:].opt()],
                    outs=[out_b[:].opt()],
                )
                g = tpool.tile([128, 2], F32, tag="g")
                nc.gpsimd.dma_start(out=g[:], in_=out_b[:])
                # fold image halves: s = g[0:64] + g[64:128]
                h = tpool.tile([64, 2], F32, tag="h")
                nc.sync.dma_start(out=h[:], in_=g[64:128, :])
                s = tpool.tile([64, 2], F32, tag="s")
                nc.vector.tensor_tensor(
                    out=s[:], in0=g[0:64, :], in1=h[:], op=mybir.AluOpType.add
                )
                # m = s0/16 ; var = s1/16 - m^2 ; tneg = bw*sqrt(var+eps) - m
                m = tpool.tile([64, 1], F32, tag="m")
                nc.vector.tensor_scalar(
                    out=m[:], in0=s[:, 0:1], scalar1=1.0 / N, scalar2=None,
                    op0=mybir.AluOpType.mult,
                )
                var = tpool.tile([64, 1], F32, tag="var")
                nc.vector.tensor_scalar(
                    out=var[:], in0=s[:, 1:2], scalar1=1.0 / N, scalar2=None,
                    op0=mybir.AluOpType.mult,
                )
                m2 = tpool.tile([64, 1], F32, tag="m2")
                nc.vector.tensor_tensor(
                    out=m2[:], in0=m[:], in1=m[:], op=mybir.AluOpType.mult
                )
                nc.vector.tensor_tensor(
                    out=var[:], in0=var[:], in1=m2[:],
                    op=mybir.AluOpType.subtract,
                )
                eps = tpool.tile([64, 1], F32, tag="eps")
                nc.vector.memset(eps[:], float(BN_EPS))
                sd = tpool.tile([64, 1], F32, tag="sd")
                nc.scalar.activation(
                    out=sd[:], in_=var[:],
                    func=mybir.ActivationFunctionType.Sqrt, bias=eps[:],
                )
                nc.vector.tensor_tensor(
                    out=sd[:], in0=sd[:], in1=bwsb[:], op=mybir.AluOpType.mult
                )
                nc.vector.tensor_tensor(
                    out=tneg[0:64, :], in0=sd[:], in1=m[:],
                    op=mybir.AluOpType.subtract,
                )
                nc.sync.dma_start(out=tneg[64:128, :], in_=tneg[0:64, :])

            def emit_phase2(it):
                for b in range(NB):
                    r0 = b * BAND
                    lo = max(r0 - 1, 0)
                    hi = min(r0 + BAND + 1, H)
                    s0 = lo - (r0 - 1)
                    nr = hi - lo
                    p0 = sp0[b % 2]
                    p1 = sp1[b % 2]
                    s3_0 = p0[:, 1 : 1 + STRIP_ROWS * WP].rearrange(
                        "p (a b) -> p a b", b=WP
                    )
                    s3_1 = p1[:, 1 : 1 + STRIP_ROWS * WP].rearrange(
                        "p (a b) -> p a b", b=WP
                    )
                    if b == NB - 1:
                        # bottom pad row on the unshifted halves
                        nc.vector.memset(s3_0[0:64, STRIP_ROWS - 1, :], 0.0)
                        nc.vector.memset(s3_1[64:128, STRIP_ROWS - 1, :], 0.0)
                    # sign strips (unshifted halves), emitted in two row
                    # chunks so the first psum group's matmuls can start
                    # before the whole band is signed
                    mid = s0 + nr // 2
                    for (ra, rb) in ((s0, mid), (mid, s0 + nr)):
                        xa, xb_ = lo + (ra - s0), lo + (rb - s0)
                        nc.scalar.activation(
                            out=s3_0[0:64, ra:rb, 1 : 1 + W],
                            in_=x16[0:64, xa:xb_, :],
                            func=mybir.ActivationFunctionType.Sign,
                            bias=tneg[0:64, :],
                        )
                        nc.scalar.activation(
                            out=s3_1[64:128, ra:rb, 1 : 1 + W],
                            in_=x16[64:128, xa:xb_, :],
                            func=mybir.ActivationFunctionType.Sign,
                            bias=tneg[64:128, :],
                        )
                    # shifted copies on the other partition half:
                    # slot s <- strip row s+1, s = 0..28; two chunks each
                    for (sa, sb) in ((0, 14), (14, STRIP_ROWS - 1)):
                        cl = (sb - sa) * WP
                        nc.scalar.dma_start(
                            out=p0[64:128, 1 + sa * WP : 1 + sa * WP + cl],
                            in_=p0[0:64, 1 + (sa + 1) * WP : 1 + (sa + 1) * WP + cl],
                        )
                        nc.scalar.dma_start(
                            out=p1[0:64, 1 + sa * WP : 1 + sa * WP + cl],
                            in_=p1[64:128, 1 + (sa + 1) * WP : 1 + (sa + 1) * WP + cl],
                        )

                    for g0 in range(0, NT, 4):
                        tiles = list(range(g0, min(g0 + 4, NT)))
                        stg = opool.tile([128, 8, W], F32, tag="stg")
                        psums = {
                            i: ppool.tile([128, 512], F32, name=f"ps{i}", tag="ps")
                            for i in tiles
                        }
                        NM = 56 if mm_variant == "n56" else MM_N
                        # pair rounds: taps (0,dx)+(1,dx) via 128-row
                        # contraction (strip + shifted strip); full array
                        # per span (img0 on psum 0:64 / col grps 0-1, img1
                        # on 64:128 / col grps 2-3).
                        for dx in range(3):
                            wdx0 = 0 if mm_variant == "samew" else dx
                            wdx1 = 0 if mm_variant == "samew" else 3 + dx
                            for i in tiles:
                                st = (2 * i) * WP + dx
                                nc.tensor.matmul(
                                    psums[i][0:64, :NM],
                                    wsb[:, wdx0, :],
                                    p0[:, st : st + NM],
                                    start=(dx == 0), stop=False,
                                    skip_group_check=True,
                                )
                                nc.tensor.matmul(
                                    psums[i][64:128, :NM],
                                    wsb[:, wdx1, :],
                                    p1[:, st : st + NM],
                                    start=(dx == 0), stop=False,
                                    skip_group_check=True,
                                )
                        # single rounds: tap (2,dx), K=64.  Two tiles share
                        # one span: tile i streams the unshifted half (row
                        # grps of that half), tile j=i+1 streams the other
                        # half via the shifted copy at st-WP; disjoint
                        # 32x32 subarray sets -> concurrent.
                        for dx in range(3):
                            last = dx == 2
                            wdxs = 0 if mm_variant == "samew" else 6 + dx
                            for k in range(0, len(tiles), 2):
                                i = tiles[k]
                                sti = (2 * i + 2) * WP + dx
                                nc.tensor.matmul(
                                    psums[i][0:64, :NM],
                                    wsb[0:64, wdxs, :],
                                    p0[0:64, sti : sti + NM],
                                    start=False, stop=last,
                                    skip_group_check=True,
                                )
                                nc.tensor.matmul(
                                    psums[i][64:128, :NM],
                                    wsb[64:128, wdxs, :],
                                    p1[64:128, sti : sti + NM],
                                    start=False, stop=last,
                                    skip_group_check=True,
                                )
                                if k + 1 < len(tiles):
                                    j = tiles[k + 1]
                                    stj = (2 * j + 1) * WP + dx
                                    nc.tensor.matmul(
                                        psums[j][0:64, :NM],
                                        wsb[64:128, wdxs, :],
                                        p0[64:128, stj : stj + NM],
                                        start=False, stop=last,
                                        skip_group_check=True,
                                    )
                                    nc.tensor.matmul(
                                        psums[j][64:128, :NM],
                                        wsb[0:64, wdxs, :],
                                        p1[0:64, stj : stj + NM],
                                        start=False, stop=last,
                                        skip_group_check=True,
                                    )
                        for k, i in enumerate(tiles):
                            ps3 = psums[i][:, :MM_N].rearrange(
                                "p (r c) -> p r c", c=WP
                            )
                            nc.vector.tensor_scalar(
                                out=stg[:, 2 * k : 2 * k + 2, :],
                                in0=ps3[:, :, 1 : 1 + W],
                                scalar1=bsb[:],
                                scalar2=None,
                                op0=mybir.AluOpType.add,
                            )
                        nc.sync.dma_start(
                            out=y[:, r0 + 2 * g0 : r0 + 2 * g0 + 2 * len(tiles), :],
                            in_=stg[:, : 2 * len(tiles), :],
                        )

            if timing is None:
                for it in range(repeat):
                    u = emit_stats(it)
                    emit_cc(it, u)
                    emit_phase2(it)
            else:
                phase, R = timing
                mvout = tpool.tile([128, 2], F32, tag="mvout")
                nc.vector.memset(mvout[:], 0.0)
                if phase != "p1":
                    nc.vector.memset(tneg[:], 0.0)
                nc.vector.memset(x16[:], 0.0)
                if phase == "iter":
                    with tc.For_i(0, R, 1):
                        u = emit_stats(0)
                        emit_phase2(0)
                elif phase == "p1":
                    with tc.For_i(0, R, 1):
                        u = emit_stats(0)
                elif phase == "p2":
                    with tc.For_i(0, R, 1):
                        emit_phase2(0)
                elif phase == "cc":
                    for it in range(R):
                        u = emit_stats(it) if it == 0 else u
                        emit_cc(it, u)
                else:
                    raise ValueError(phase)
                nc.sync.dma_start(out=tout[:], in_=mvout[:])
    nc.compile()
    return nc


_cache = {}


def _get(name, builder):
    if name not in _cache:
        _cache[name] = builder()
    return _cache[name]


def _prep_inputs(bn_weight, bn_bias, conv_weight, conv_bias):
    # lhsT layouts; w9[cin, dy, dx, cout]
    w9 = conv_weight.transpose(1, 2, 3, 0).reshape(C, 3, 3, C)
    wts = np.empty((128, 9, C), np.float16)
    for dx in range(3):
        wts[0:64, dx, :] = w9[:, 0, dx, :]       # img0 pair: strip half
        wts[64:128, dx, :] = w9[:, 1, dx, :]     # img0 pair: shifted half
        wts[0:64, 3 + dx, :] = w9[:, 1, dx, :]   # img1 pair: shifted half
        wts[64:128, 3 + dx, :] = w9[:, 0, dx, :]  # img1 pair: strip half
        wts[0:64, 6 + dx, :] = w9[:, 2, dx, :]   # singles
        wts[64:128, 6 + dx, :] = w9[:, 2, dx, :]
    bwv = (bn_bias.astype(np.float64) / bn_weight.astype(np.float64)).astype(
        np.float32
    )[:, None]
    cb = np.tile(conv_bias.astype(np.float32), 2)[:, None]
    return wts, bwv, cb


def kernel(x, bn_weight, bn_bias, conv_weight, conv_bias):
    x = np.ascontiguousarray(np.asarray(x), dtype=np.float32)
    bn_weight = np.asarray(bn_weight, dtype=np.float32)
    bn_bias = np.asarray(bn_bias, dtype=np.float32)
    conv_weight = np.asarray(conv_weight, dtype=np.float32)
    conv_bias = np.asarray(conv_bias, dtype=np.float32)

    wts, bwv, cb = _prep_inputs(bn_weight, bn_bias, conv_weight, conv_bias)

    ipc = N // N_CORES
    nc = _get("fused", build_fused_nc)
    in_maps = [
        {
            "x_s": x[ipc * c : ipc * (c + 1)].reshape(128, H * W),
            "wts": wts,
            "bw": bwv,
            "cbias": cb,
        }
        for c in range(N_CORES)
    ]
    res = run_bass_kernel_spmd(nc, in_maps, list(range(N_CORES))).results
    out = np.concatenate(
        [res[c]["y"].reshape(ipc, C, H, W) for c in range(N_CORES)], axis=0
    )
    return out
